# revision 10
# baseline (speedup 1.0000x reference)
"""GANLoss kernel for Trainium2: out = -sum_i prob[i, target[i]] * reward[i].

Shapes: prob (8192, 32000) f32, target (8192,) int64, reward (8192,) f32.
Sharding: rows split across 8 NeuronCores (1024 rows/core).

Strategy: the loss touches only one element per row, so instead of streaming
the full 131 MB/core shard we dma_gather the 256-float (1 KB) chunk that
contains each row's target element (4 gather calls x 256 indices per core,
~1 MB read/core), then select the element with an iota/is_equal mask fused
with the reward multiply, and reduce on the vector engine. Each core emits
a [128, 4] tile of partial sums; the host sums them and negates.
"""

import numpy as np

N, C = 8192, 32000
N_CORES = 8
ROWS_PER_CORE = N // N_CORES          # 1024
N_GATHER = 4                          # gather calls per core
ROWS_PER_CALL = 256                   # idxs per gather call
ELEM = 256                            # f32 per gathered chunk (1 KB)
CHUNKS_PER_ROW = C // ELEM            # 125; max idx 255*125+124 = 31999 < 2^15

_cached = None


def _build_bass():
    import concourse.bacc as bacc
    import concourse.mybir as mybir

    f32 = mybir.dt.float32
    i16 = mybir.dt.int16

    nc = bacc.Bacc(num_swdge_queues=4)
    prob_d = nc.declare_dram_parameter("prob", [ROWS_PER_CORE, C], f32, isOutput=False)
    gidx_d = nc.declare_dram_parameter("gidx", [128, 16 * N_GATHER], i16, isOutput=False)
    offs_d = nc.declare_dram_parameter("offs", [128, 2 * N_GATHER], f32, isOutput=False)
    rew_d = nc.declare_dram_parameter("rew", [128, 2 * N_GATHER], f32, isOutput=False)
    out_d = nc.declare_dram_parameter("out", [128, N_GATHER], f32, isOutput=True)

    with (
        nc.sbuf_tensor([128, 16 * N_GATHER], i16) as idx_sb,
        nc.sbuf_tensor([128, 2 * N_GATHER], f32) as offs_sb,
        nc.sbuf_tensor([128, 2 * N_GATHER], f32) as rew_sb,
        nc.sbuf_tensor([128, ELEM], f32) as iota_sb,
        nc.sbuf_tensor([128, N_GATHER, 2, ELEM], f32) as gath_sb,
        nc.sbuf_tensor([128, 2 * ELEM], f32) as mask_sb,
        nc.sbuf_tensor([128, 2 * ELEM], f32) as prod_sb,
        nc.sbuf_tensor([128, N_GATHER], f32) as out_sb,
        nc.semaphore("in_sem") as in_sem,
        nc.semaphore("gs0") as gs0,
        nc.semaphore("gs1") as gs1,
        nc.semaphore("gs2") as gs2,
        nc.semaphore("gs3") as gs3,
        nc.semaphore("comp_sem") as comp_sem,
        nc.semaphore("ts_sem") as ts_sem,
        nc.semaphore("iota_sem") as iota_sem,
        nc.Block() as block,
    ):
        gsems = [gs0, gs1, gs2, gs3]

        @block.gpsimd
        def _(g):
            g.iota(
                iota_sb[:],
                pattern=[[1, ELEM]],
                base=0,
                channel_multiplier=0,
                allow_small_or_imprecise_dtypes=True,
            ).then_inc(iota_sem, 1)
            g.dma_start(idx_sb[:], gidx_d[:]).then_inc(in_sem, 16)
            g.dma_start(offs_sb[:], offs_d[:]).then_inc(in_sem, 16)
            g.dma_start(rew_sb[:], rew_d[:]).then_inc(in_sem, 16)
            g.wait_ge(in_sem, 48)
            for gi in range(N_GATHER):
                src = prob_d[ROWS_PER_CALL * gi : ROWS_PER_CALL * (gi + 1), :].rearrange(
                    "r (c e) -> (r c) e", e=ELEM
                )
                g.dma_gather(
                    gath_sb[:, gi],
                    src,
                    idx_sb[:, 16 * gi : 16 * (gi + 1)],
                    num_idxs=ROWS_PER_CALL,
                    num_idxs_reg=ROWS_PER_CALL,
                    elem_size=ELEM,
                    queue_num=gi,
                ).then_inc(gsems[gi], 16)
            g.wait_ge(comp_sem, N_GATHER)
            g.dma_start(out_d[:], out_sb[:]).then_inc(in_sem, 16)
            g.wait_ge(in_sem, 64)

        @block.vector
        def _(v):
            v.wait_ge(iota_sem, 1)
            v.wait_ge(in_sem, 48)
            for gi in range(N_GATHER):
                if gi > 0:
                    v.wait_ge(ts_sem, 3 * gi)  # prior mult done: mask free
                    v.wait_ge(comp_sem, gi)  # prior reduce done: prod free
                # maskrew[p, c*ELEM + w] = (w == t%ELEM) * reward  for row 256gi+128c+p
                v.tensor_scalar(
                    mask_sb[:, 0:ELEM],
                    iota_sb[:],
                    offs_sb[:, 2 * gi : 2 * gi + 1],
                    rew_sb[:, 2 * gi : 2 * gi + 1],
                    op0=mybir.AluOpType.is_equal,
                    op1=mybir.AluOpType.mult,
                ).then_inc(ts_sem, 1)
                v.tensor_scalar(
                    mask_sb[:, ELEM : 2 * ELEM],
                    iota_sb[:],
                    offs_sb[:, 2 * gi + 1 : 2 * gi + 2],
                    rew_sb[:, 2 * gi + 1 : 2 * gi + 2],
                    op0=mybir.AluOpType.is_equal,
                    op1=mybir.AluOpType.mult,
                ).then_inc(ts_sem, 1)
                v.wait_ge(ts_sem, 3 * gi + 2)
                v.wait_ge(gsems[gi], 16)
                v.tensor_mul(
                    prod_sb[:],
                    gath_sb[:, gi].rearrange("p a b -> p (a b)"),
                    mask_sb[:],
                ).then_inc(ts_sem, 1)
                v.wait_ge(ts_sem, 3 * gi + 3)
                v.tensor_reduce(
                    out_sb[:, gi : gi + 1],
                    prod_sb[:],
                    axis=mybir.AxisListType.X,
                    op=mybir.AluOpType.add,
                ).then_inc(comp_sem, 1)

    nc.compile()
    return nc


def _shard_host_inputs(prob, target, reward):
    """Per-core in_maps: prob shard + precomputed gather indices/offsets."""
    t_all = np.asarray(target).astype(np.int64)
    r_all = np.asarray(reward).astype(np.float32)
    prob = np.ascontiguousarray(np.asarray(prob, dtype=np.float32))
    in_maps = []
    loc = np.arange(ROWS_PER_CALL)
    for core in range(N_CORES):
        base = core * ROWS_PER_CORE
        t = t_all[base : base + ROWS_PER_CORE]
        r = r_all[base : base + ROWS_PER_CORE]
        chunk = (t // ELEM).astype(np.int64)
        off = (t % ELEM).astype(np.float32)
        gidx16 = np.zeros((16, 16 * N_GATHER), np.int16)
        offs = np.zeros((128, 2 * N_GATHER), np.float32)
        rew = np.zeros((128, 2 * N_GATHER), np.float32)
        for g in range(N_GATHER):
            rb = ROWS_PER_CALL * g
            idxv = loc * CHUNKS_PER_ROW + chunk[rb + loc]
            gidx16[loc % 16, 16 * g + loc // 16] = idxv.astype(np.int16)
            for ci in range(2):
                offs[:, 2 * g + ci] = off[rb + 128 * ci : rb + 128 * ci + 128]
                rew[:, 2 * g + ci] = r[rb + 128 * ci : rb + 128 * ci + 128]
        # the 8 GPSIMD cores each read their own 16-partition copy
        gidx = np.tile(gidx16, (8, 1))
        in_maps.append(
            {
                "prob": prob[base : base + ROWS_PER_CORE],
                "gidx": gidx,
                "offs": offs,
                "rew": rew,
            }
        )
    return in_maps


def kernel(prob, target, reward):
    global _cached
    from concourse.bass_utils import run_bass_kernel_spmd

    if _cached is None:
        _cached = _build_bass()
    nc = _cached
    in_maps = _shard_host_inputs(prob, target, reward)
    res = run_bass_kernel_spmd(nc, in_maps, list(range(N_CORES)))
    total = np.float64(0.0)
    for core_out in res.results:
        total += np.asarray(core_out["out"], dtype=np.float64).sum()
    return np.float32(-total)
